# Initial kernel scaffold
#
"""Multi-head attention (B=4, S=1024, D=1024, H=16) on 8 Trainium2 NeuronCores.

Sharding: core c handles batch b = c//2 and head-group g = c%2 (8 of 16 heads).
Each core computes its heads' Q/K/V projections, attention, and a partial
output projection against its 512 rows of W_o.T; the host sums the two
partials per batch and adds b_o.

Device layout tricks:
- Q/K are produced transposed ([dh, s]) straight out of the projection
  matmuls, so attention scores come out as scoresT[sk, sq] with key
  positions on partitions. The padding mask is then a per-partition bias
  on the fused exp activation (exp(0.125*s - 1e6) == 0 in fp32).
- V is produced in [sk, dh] layout with an extra "ones" column per head, so
  one matmul accumulation yields both the attention numerator (partitions
  0..63) and the softmax denominator (partition 64). A single small
  SBUF->SBUF DMA gathers the 16 denominator rows onto partitions 64..71
  (one per head) for the reciprocal-broadcast matmul.
- 1/denominator is broadcast across the 64 rows of each head with a tiny
  selector matmul on the PE, then folded into the merged heads with one
  elementwise multiply before the output projection.
- All matmuls run as float32r (full fp32 operands, ~4x fp32 speed).
"""

import numpy as np

import concourse.bacc as bacc
import concourse.tile as tile
import concourse.mybir as mybir
from concourse.bass_utils import run_bass_kernel_spmd

F32 = mybir.dt.float32
F32R = mybir.dt.float32r
EXP = mybir.ActivationFunctionType.Exp

B, S, D, H = 4, 1024, 1024, 16
DH = D // H            # 64
G = H // 2             # 8 heads per core
GC = G * DH            # 512 output cols per core
NEG = -1000000.0
P = 128
NDC = D // P           # 8 contraction chunks
NTH = 4                # head-pair tiles (GC/128)


def r(ap):
    return ap.bitcast(F32R)


def build(SKT, with_bq, with_bk, with_bv):
    """Build the SPMD program. SKT = number of 128-row key tiles computed."""
    nc = bacc.Bacc(None, target_bir_lowering=False, debug=False)

    xq = nc.dram_tensor("xq", [D, S], F32R, kind="ExternalInput")    # queries[b].T
    xk = nc.dram_tensor("xk", [D, S], F32R, kind="ExternalInput")    # keys[b].T
    xv = nc.dram_tensor("xv", [D, S], F32R, kind="ExternalInput")    # values[b].T
    wq = nc.dram_tensor("wq", [D, GC], F32R, kind="ExternalInput")   # W_q.T slice
    wk = nc.dram_tensor("wk", [D, GC], F32R, kind="ExternalInput")
    wv = nc.dram_tensor("wv", [D, GC], F32R, kind="ExternalInput")
    wo = nc.dram_tensor("wo", [GC, D], F32R, kind="ExternalInput")   # W_o.T rows
    mkb = nc.dram_tensor("mkb", [S], F32, kind="ExternalInput")     # 0 / -1e6
    esel = nc.dram_tensor("esel", [2, P], F32R, kind="ExternalInput")
    bq = nc.dram_tensor("bq", [GC], F32, kind="ExternalInput")
    bk = nc.dram_tensor("bk", [GC], F32, kind="ExternalInput")
    bv = nc.dram_tensor("bv", [GC], F32R, kind="ExternalInput")
    out = nc.dram_tensor("out", [S, D], F32, kind="ExternalOutput")

    SK = SKT * P

    VW = DH + 1              # 65: per-head V slot width (64 V cols + ones col)

    with tile.TileContext(nc) as tc:
        with tc.tile_pool(name="persist", bufs=1) as persist, \
             tc.tile_pool(name="cst", bufs=1) as cst:
            qts = [persist.tile([P, S], F32R, tag=f"qt{i}", name=f"qt{i}")
                   for i in range(NTH)]                           # QT[dh, sq]
            kts = [persist.tile([P, SK], F32R, tag=f"kt{i}", name=f"kt{i}")
                   for i in range(NTH)]                           # KT[dh, sk]
            vp = persist.tile([P, SKT, G, VW], F32R, tag="vp")    # V + ones cols
            mgs = [persist.tile([P, S], F32R, tag=f"mg{i}", name=f"mg{i}")
                   for i in range(NTH)]                           # merged numerators


            mb = cst.tile([P, SKT], F32, tag="mb")
            # pair selector rows live at partitions 64..65 to line up with
            # the gathered denominator/reciprocal rows.
            es = cst.tile([P, P], F32R, tag="es")
            if with_bq:
                bq_sb = cst.tile([P, NTH], F32, tag="bq")
                nc.sync.dma_start(out=bq_sb[:], in_=bq.rearrange("(t p) -> p t", p=P))
            if with_bk:
                bk_sb = cst.tile([P, NTH], F32, tag="bk")
                nc.sync.dma_start(out=bk_sb[:], in_=bk.rearrange("(t p) -> p t", p=P))
            if with_bv:
                bv_sb = cst.tile([1, GC], F32R, tag="bv")
                nc.sync.dma_start(out=bv_sb[:], in_=bv[None, :])
                ones1f = cst.tile([1, P], F32, tag="ones1f")
                nc.vector.memset(ones1f[:], 1.0)
                ones1 = cst.tile([1, P], F32R, tag="ones1")
                nc.vector.tensor_copy(ones1[:], ones1f[:])

            # ACT exp-table preload: dummy activation so the ~2.7us
            # ACT_TABLE_LOAD happens during the startup DMA wait.
            wtb = cst.tile([1, 16], F32, tag="wtb")
            wtb0 = cst.tile([1, 1], F32, tag="wtb0")
            nc.vector.memset(wtb[:], 0.0)
            nc.vector.memset(wtb0[:], 0.0)
            nc.scalar.activation(wtb[:], wtb[:], EXP, bias=wtb0[:], scale=1.0)

            # V slots: ones column at position DH of every head slot.
            # (memset a plain-f32 staging tile, then DVE-copy per head: the
            # copy casts/rounds to f32r, which the BIR verifier requires for
            # anything feeding an fp32r matmul.)
            scrf = cst.tile([P, 640], F32, tag="scrf")
            nc.vector.memset(scrf[:], 0.001)
            scr = cst.tile([P, 640], F32R, tag="scr")
            nc.vector.tensor_copy(scr[:], scrf[:])

            onesw = cst.tile([P, SKT], F32, tag="onesw")
            nc.vector.memset(onesw[:], 1.0)
            for h in range(G):
                nc.vector.tensor_copy(vp[:, :, h, DH], onesw[:])

            # ---- Phase A: projections ----
            # First-chunk operands live in their own small tiles so the very
            # first matmul only waits on two small DMAs (tile-granularity
            # dependency tracking would otherwise stall it on the bulk loads).
            with tc.tile_pool(name="wts", bufs=1) as wts, \
                 tc.tile_pool(name="xs", bufs=2) as xs, \
                 tc.tile_pool(name="psA", bufs=8, space="PSUM") as psA:
                pswu = psA.tile([P, 512], F32, tag="psA", name="pswu")
                for i in range(20):
                    nc.tensor.matmul(pswu[:], r(scr[:, 0:128]), r(scr[:, 128:640]),
                                     start=(i == 0), stop=(i == 19))

                wq0 = wts.tile([P, GC], F32R, tag="wq0")
                nc.sync.dma_start(out=wq0[:], in_=wq[0:P, :])
                wqr = wts.tile([P, NDC - 1, GC], F32R, tag="wqr")
                for c in range(1, NDC):
                    nc.sync.dma_start(out=wqr[:, c - 1, :], in_=wq[c * P:(c + 1) * P, :])
                wk_sb = wts.tile([P, NDC, GC], F32R, tag="wk")
                wv_sb = wts.tile([P, NDC, GC], F32R, tag="wv")

                def wq_at(c):
                    return wq0[:] if c == 0 else wqr[:, c - 1, :]

                # QT[128*t:+128, sq] = sum_c wq[c, t-slice].T @ xq[c, half]
                for half in range(2):
                    qsl = slice(half * 512, (half + 1) * 512)
                    pss = [psA.tile([P, 512], F32, tag="psA", name=f"psA_{half}_{t}")
                           for t in range(NTH)]
                    xt0 = xs.tile([P, 512], F32R, tag="xq0")
                    nc.sync.dma_start(out=xt0[:], in_=xq[0:P, qsl])
                    xtr = xs.tile([P, NDC - 1, 512], F32R, tag="xqr")
                    for c in range(1, NDC):
                        nc.sync.dma_start(out=xtr[:, c - 1, :],
                                          in_=xq[c * P:(c + 1) * P, qsl])
                    for c in range(NDC):
                        rhs = xt0[:] if c == 0 else xtr[:, c - 1, :]
                        for t in range(NTH):
                            nc.tensor.matmul(
                                pss[t][:], r(wq_at(c)[:, t * P:(t + 1) * P]), r(rhs),
                                start=(c == 0), stop=(c == NDC - 1))
                    for t in range(NTH):
                        if with_bq:
                            nc.vector.tensor_scalar_add(
                                qts[t][:, qsl], pss[t][:], bq_sb[:, t:t + 1])
                        else:
                            nc.vector.tensor_copy(qts[t][:, qsl], pss[t][:])

                for c in range(NDC):
                    nc.sync.dma_start(out=wk_sb[:, c, :], in_=wk[c * P:(c + 1) * P, :])
                # V[sk_tile, dh'] = sum_c xv[c, sk_tile].T @ wv[c, :]  (+ b_v)
                for c in range(NDC):
                    nc.sync.dma_start(out=wv_sb[:, c, :], in_=wv[c * P:(c + 1) * P, :])
                psvs = [psA.tile([P, GC], F32, tag="psA", name=f"psv{st}")
                        for st in range(SKT)]
                for c in range(NDC):
                    xt = xs.tile([P, SK], F32R, tag="xv")
                    nc.sync.dma_start(out=xt[:], in_=xv[c * P:(c + 1) * P, 0:SK])
                    for st in range(SKT):
                        nc.tensor.matmul(psvs[st][:], r(xt[:, st * P:(st + 1) * P]),
                                         r(wv_sb[:, c, :]), start=(c == 0),
                                         stop=(c == NDC - 1 and not with_bv))
                for st in range(SKT):
                    if with_bv:
                        nc.tensor.matmul(psvs[st][:], r(ones1[:]), r(bv_sb[:]),
                                         start=False, stop=True)
                    # scatter per-head 64-col slices into the 65-wide slots
                    nc.vector.tensor_copy(
                        vp[:, st, :, 0:DH],
                        psvs[st][:].rearrange("p (g d) -> p g d", g=G))

                kgroups = [(s0, min(512, SK - s0)) for s0 in range(0, SK, 512)]
                xkts = []
                for gi, (s0, w) in enumerate(kgroups):
                    xtr = xs.tile([P, NDC, 512], F32R, tag="xkr", bufs=2,
                                  name=f"xkr{gi}")
                    for c in range(NDC):
                        nc.sync.dma_start(out=xtr[:, c, :w],
                                          in_=xk[c * P:(c + 1) * P, s0:s0 + w])
                    xkts.append(xtr)
                # t-outer: finish pair 0's K tile first so its scores/exp
                # pipeline starts as early as possible.
                for t in range(NTH):
                    for gi, (s0, w) in enumerate(kgroups):
                        psk = psA.tile([P, 512], F32, tag="psA", name=f"psK_{gi}_{t}")
                        for c in range(NDC):
                            nc.tensor.matmul(
                                psk[:, :w], r(wk_sb[:, c, t * P:(t + 1) * P]),
                                r(xkts[gi][:, c, :w]),
                                start=(c == 0), stop=(c == NDC - 1))
                        if with_bk:
                            nc.vector.tensor_scalar_add(
                                kts[t][:, s0:s0 + w], psk[:, :w], bk_sb[:, t:t + 1])
                        else:
                            nc.vector.tensor_copy(
                                kts[t][:, s0:s0 + w], psk[:, :w])
            nc.sync.dma_start(out=mb[:], in_=mkb[: SK].rearrange("(t p) -> p t", p=P))
            nc.sync.dma_start(out=es[64:66, :], in_=esel[:, :])

            # ---- Phase B: attention per head pair ----
            # (wo prefetch kicks off here so its DMA overlaps attention compute)
            wop_cm = tc.tile_pool(name="wop", bufs=1)
            wop = wop_cm.__enter__()
            wo_sb = wop.tile([P, NTH, D], F32R, tag="wo")
            for c in range(NTH):
                nc.sync.dma_start(out=wo_sb[:, c, :], in_=wo[c * P:(c + 1) * P, :])
            with tc.tile_pool(name="pp", bufs=2 if SKT <= 6 else 1) as pp, \
                 tc.tile_pool(name="dpool", bufs=2) as dpool, \
                 tc.tile_pool(name="psS", bufs=3, space="PSUM") as psS, \
                 tc.tile_pool(name="psV", bufs=1, space="PSUM") as psV, \
                 tc.tile_pool(name="psR", bufs=1, space="PSUM") as psR:
                for th in range(NTH):
                    he, ho = 2 * th, 2 * th + 1
                    # full [sk, sq=1024] p tiles per head; one fused exp per
                    # (head, sk tile) amortizes ACT's ~352-cycle op overhead.
                    pte = pp.tile([P, SKT, S], F32R, tag="pe")
                    pto = pp.tile([P, SKT, S], F32R, tag="po")
                    for st in range(SKT):
                        ksl = slice(st * P, (st + 1) * P)
                        pse = psS.tile([P, S], F32, tag="psS")
                        pso = psS.tile([P, S], F32, tag="psS", name=f"psSo_{th}_{st}")
                        for half in range(2):
                            qsl = slice(half * 512, (half + 1) * 512)
                            nc.tensor.matmul(pse[:, qsl], r(kts[th][0:64, ksl]),
                                             r(qts[th][0:64, qsl]), start=True, stop=True)
                            nc.tensor.matmul(pso[:, qsl], r(kts[th][64:128, ksl]),
                                             r(qts[th][64:128, qsl]), start=True, stop=True)
                        nc.scalar.activation(pte[:, st, :], pse[:], EXP,
                                             bias=mb[:, st:st + 1], scale=0.125)
                        nc.scalar.activation(pto[:, st, :], pso[:], EXP,
                                             bias=mb[:, st:st + 1], scale=0.125)
                    # attnV per half (both heads), then normalize that half
                    # while the other half's attnV still runs. Reciprocal is
                    # lane-parallel, so spread each half's 1024 denominators
                    # over 8 partitions (gather), recip, and scatter back.
                    dst_t = dpool.tile([P, 2, 2, 512], F32, tag="dstp")
                    rsg_t = dpool.tile([P, 2, 128], F32, tag="rsgp")
                    rcp_t = dpool.tile([P, 2, 128], F32R, tag="rcpp")
                    rst_t = dpool.tile([P, 2, 512], F32R, tag="rstp")
                    for half in range(2):
                        qsl = slice(half * 512, (half + 1) * 512)
                        for hi, (h, pt) in enumerate(((he, pte), (ho, pto))):
                            nv = psV.tile([P, 512], F32, tag="psV")
                            for st in range(SKT):
                                nc.tensor.matmul(nv[0:DH + 1, :], r(vp[:, st, h, :]),
                                                 r(pt[:, st, qsl]),
                                                 start=(st == 0), stop=(st == SKT - 1))
                            nc.vector.tensor_copy(
                                mgs[th][64 * hi:64 * hi + 64, qsl], nv[0:64, :])
                            nc.vector.tensor_copy(dst_t[64:65, hi, half, :],
                                                  nv[64:65, :])
                        for hi2 in range(2):
                            nc.sync.dma_start(
                                out=rsg_t[64 + 4 * hi2:68 + 4 * hi2, half, :],
                                in_=dst_t[64:65, hi2, half, :])
                        with nc.allow_low_precision("softmax denom recip at fp32r"):
                            nc.vector.reciprocal(rcp_t[64:72, half, :],
                                                 rsg_t[64:72, half, :])
                        nc.sync.dma_start(out=rst_t[64:66, half, :],
                                          in_=rcp_t[64:72, half, :])
                        pr = psR.tile([P, 512], F32, tag="psR")
                        nc.tensor.matmul(pr[:], r(es[64:66, :]),
                                         r(rst_t[64:66, half, :]), start=True, stop=True)
                        nc.vector.tensor_mul(mgs[th][:, qsl], mgs[th][:, qsl], pr[:])

            # ---- Phase C: normalize + output projection ----
            with tc.tile_pool(name="ot", bufs=3) as ot, \
                 tc.tile_pool(name="psO", bufs=4, space="PSUM") as psO:
                for qt_i in range(8):
                    sqsl = slice(qt_i * P, (qt_i + 1) * P)
                    for oh in range(2):
                        osl = slice(oh * 512, (oh + 1) * 512)
                        pso = psO.tile([P, 512], F32, tag="psO")
                        for c in range(NTH):
                            nc.tensor.matmul(pso[:], r(mgs[c][:, sqsl]),
                                             r(wo_sb[:, c, osl]),
                                             start=(c == 0), stop=(c == NTH - 1))
                        ob = ot.tile([P, 512], F32, tag="ob")
                        if (qt_i + oh) % 2 == 0:
                            nc.vector.tensor_copy(ob[:], pso[:])
                        else:
                            nc.scalar.copy(ob[:], pso[:])
                        nc.sync.dma_start(out=out[sqsl, osl], in_=ob[:])
            wop_cm.__exit__(None, None, None)

    nc.finalize()
    return nc


_CACHE = {}


def kernel(**inputs):
    queries = np.asarray(inputs["queries"], np.float32)
    keys = np.asarray(inputs["keys"], np.float32)
    values = np.asarray(inputs["values"], np.float32)
    valid_lens = np.asarray(inputs["valid_lens"], np.int32)
    W_q = np.asarray(inputs["W_q"], np.float32)
    W_k = np.asarray(inputs["W_k"], np.float32)
    W_v = np.asarray(inputs["W_v"], np.float32)
    W_o = np.asarray(inputs["W_o"], np.float32)
    b_q = np.asarray(inputs["b_q"], np.float32)
    b_k = np.asarray(inputs["b_k"], np.float32)
    b_v = np.asarray(inputs["b_v"], np.float32)
    b_o = np.asarray(inputs["b_o"], np.float32)

    maxv = int(valid_lens.max())
    SKT = max(1, min(8, -(-maxv // P)))
    with_bq, with_bk, with_bv = bool(b_q.any()), bool(b_k.any()), bool(b_v.any())

    key = (SKT, with_bq, with_bk, with_bv)
    if key not in _CACHE:
        _CACHE[key] = build(SKT, with_bq, with_bk, with_bv)
    nc = _CACHE[key]

    esel = np.zeros((2, P), np.float32)
    esel[0, 0:DH] = 1.0
    esel[1, DH:2 * DH] = 1.0

    col = np.arange(S)
    in_maps = []
    for c in range(8):
        b, g = c // 2, c % 2
        gsl = slice(g * GC, (g + 1) * GC)
        mkb = np.where(col < valid_lens[b], 0.0, NEG).astype(np.float32)
        in_maps.append({
            "xq": np.ascontiguousarray(queries[b].T),
            "xk": np.ascontiguousarray(keys[b].T),
            "xv": np.ascontiguousarray(values[b].T),
            "wq": np.ascontiguousarray(W_q.T[:, gsl]),
            "wk": np.ascontiguousarray(W_k.T[:, gsl]),
            "wv": np.ascontiguousarray(W_v.T[:, gsl]),
            "wo": np.ascontiguousarray(W_o.T[gsl, :]),
            "mkb": mkb,
            "esel": esel,
            "bq": np.ascontiguousarray(b_q[gsl]),
            "bk": np.ascontiguousarray(b_k[gsl]),
            "bv": np.ascontiguousarray(b_v[gsl]),
        })

    res = run_bass_kernel_spmd(nc, in_maps, list(range(8)))
    final = np.empty((B, S, D), np.float32)
    for b in range(B):
        final[b] = res.results[2 * b]["out"] + res.results[2 * b + 1]["out"] + b_o
    return final



# revision 54
# speedup vs baseline: 1.1603x; 1.1603x over previous
"""Multi-head attention (B=4, S=1024, D=1024, H=16) on 8 Trainium2 NeuronCores.

Sharding: core c handles batch b = c//2 and head-group g = c%2 (8 of 16 heads).
Each core computes its heads' Q/K/V projections, attention, and a partial
output projection against its 512 rows of W_o.T; the host sums the two
partials per batch and adds b_o.

v2 layout/scheduling:
- All matmul operands are fp16 (PSUM accumulation stays fp32): halves the
  ~19MB/core input DMA so the projection phase is PE-bound, at the same
  1 column/cycle PE rate as float32r.
- Head-pair software pipelining: pair t's scores/exp/attnV run while pair
  t+1 is still projecting, so the ACT engine's exp stream hides under PE
  work instead of gating a separate attention phase. Score st-groups are
  woven with projection/attnV filler matmuls to pace the PE queue at the
  ACT rate and avoid head-of-line PSUM stalls.
- Key-padding mask is applied by zeroing masked V rows AND the masked rows
  of the per-head "ones" denominator column (exp of a masked score is
  computed but contributes to neither numerator nor denominator). This
  frees exp from the per-tile bias operand and any mask scheduling.
- Q/K are produced transposed ([dh, s]) so scores come out [sk, sq]; V
  carries a ones column so one accumulation yields numerator + softmax
  denominator; 1/denominator rows feed a tiny selector matmul that
  broadcasts them across each head's 64 rows.
"""

import itertools

import numpy as np

import concourse.bacc as bacc
import concourse.tile as tile
import concourse.mybir as mybir
from concourse.bass_utils import run_bass_kernel_spmd
from concourse.dve_ops import RECIP_APPROX_FAST_CONSTS, RECIPROCAL_APPROX_FAST

F32 = mybir.dt.float32
F32R = mybir.dt.float32r
F16 = mybir.dt.float16
EXP = mybir.ActivationFunctionType.Exp

B, S, D, H = 4, 1024, 1024, 16
DH = D // H            # 64
G = H // 2             # 8 heads per core
GC = G * DH            # 512 output cols per core
P = 128
NDC = D // P           # 8 contraction chunks
NTH = 4                # head-pair tiles (GC/128)
VW = DH + 1            # 65: per-head V slot width (64 V cols + ones col)


def build(SKT, with_bq, with_bk, with_bv):
    nc = bacc.Bacc(None, target_bir_lowering=False, debug=False)
    SK = SKT * P

    xq = nc.dram_tensor("xq", [2, P, NDC, 512], F16, kind="ExternalInput")
    xk = nc.dram_tensor("xk", [P, NDC, SK], F16, kind="ExternalInput")
    xv = nc.dram_tensor("xv", [P, NDC, SK], F16, kind="ExternalInput")
    wq = nc.dram_tensor("wq", [NTH, P, NDC, P], F16, kind="ExternalInput")
    wk = nc.dram_tensor("wk", [NTH, P, NDC, P], F16, kind="ExternalInput")
    wv = nc.dram_tensor("wv", [P, NDC, GC], F16, kind="ExternalInput")
    wo = nc.dram_tensor("wo", [P, NTH, D], F16, kind="ExternalInput")
    vmk = nc.dram_tensor("vmk", [P, SKT], F32, kind="ExternalInput")
    esel = nc.dram_tensor("esel", [2, P], F16, kind="ExternalInput")
    bq = nc.dram_tensor("bq", [GC], F32, kind="ExternalInput")
    bk = nc.dram_tensor("bk", [GC], F32, kind="ExternalInput")
    bv = nc.dram_tensor("bv", [GC], F16, kind="ExternalInput")
    out = nc.dram_tensor("out", [S, D], F32, kind="ExternalOutput")

    with tile.TileContext(nc) as tc:
        with tc.tile_pool(name="persist", bufs=1) as persist, \
             tc.tile_pool(name="cst", bufs=1) as cst, \
             tc.tile_pool(name="pp", bufs=3) as pp, \
             tc.tile_pool(name="npool", bufs=2) as npool, \
             tc.tile_pool(name="ot", bufs=3) as ot, \
             tc.tile_pool(name="psP", bufs=2, space="PSUM") as psP, \
             tc.tile_pool(name="psS", bufs=2, space="PSUM") as psS, \
             tc.tile_pool(name="psV", bufs=2, space="PSUM") as psV:

            qts = [persist.tile([P, S], F16, tag=f"qt{i}", name=f"qt{i}")
                   for i in range(NTH)]                      # QT[dh, sq]
            kts = [persist.tile([P, SK], F16, tag=f"kt{i}", name=f"kt{i}")
                   for i in range(NTH)]                      # KT[dh, sk]
            vp = persist.tile([P, SKT, G, VW], F16, tag="vp")
            mgs = [persist.tile([P, S], F16, tag=f"mg{i}", name=f"mg{i}")
                   for i in range(NTH)]                      # merged numerators
            xq_sb = persist.tile([P, 2, NDC, 512], F16, tag="xq")
            xk_sb = persist.tile([P, NDC, SK], F16, tag="xk")
            xv_sb = persist.tile([P, NDC, SK], F16, tag="xv")
            wq_sb = persist.tile([P, NTH, NDC, P], F16, tag="wq")
            wk_sb = persist.tile([P, NTH, NDC, P], F16, tag="wk")
            wv_sb = persist.tile([P, NDC, GC], F16, tag="wv")
            wo_sb = persist.tile([P, NTH, D], F16, tag="wo")

            vm = cst.tile([P, SKT], F32, tag="vm")           # 1/0 key mask
            es = cst.tile([P, P], F16, tag="es")             # head selector rows

            # ---- input DMAs, in just-in-time consumption order ----
            nc.sync.dma_start(out=vm[:], in_=vmk[:, :])
            nc.sync.dma_start(out=es[64:66, :], in_=esel[:, :])
            if with_bq:
                bq_sb = cst.tile([P, NTH], F32, tag="bq")
                nc.sync.dma_start(out=bq_sb[:], in_=bq.rearrange("(t p) -> p t", p=P))
            if with_bk:
                bk_sb = cst.tile([P, NTH], F32, tag="bk")
                nc.sync.dma_start(out=bk_sb[:], in_=bk.rearrange("(t p) -> p t", p=P))
            if with_bv:
                bv_sb = cst.tile([1, GC], F16, tag="bv")
                nc.sync.dma_start(out=bv_sb[:], in_=bv[None, :])
                ones1 = cst.tile([1, P], F16, tag="ones1")
                nc.vector.memset(ones1[:], 1.0)
            nc.sync.dma_start(out=wq_sb[:, 0], in_=wq[0])
            for c in range(NDC):
                nc.sync.dma_start(out=xq_sb[:, 0, c], in_=xq[0, :, c])
            for c in range(NDC):
                nc.sync.dma_start(out=xq_sb[:, 1, c], in_=xq[1, :, c])
            nc.sync.dma_start(out=wk_sb[:, 0], in_=wk[0])
            kgroups = [(s0, min(512, SK - s0)) for s0 in range(0, SK, 512)]
            for c in range(NDC):
                s0, w = kgroups[0]
                nc.sync.dma_start(out=xk_sb[:, c, s0:s0 + w],
                                  in_=xk[:, c, s0:s0 + w])
            # R0's fillers (Q1/K1) need these before xk's remaining groups
            nc.sync.dma_start(out=wq_sb[:, 1], in_=wq[1])
            nc.sync.dma_start(out=wk_sb[:, 1], in_=wk[1])
            for s0, w in kgroups[1:]:
                for c in range(NDC):
                    nc.sync.dma_start(out=xk_sb[:, c, s0:s0 + w],
                                      in_=xk[:, c, s0:s0 + w])
            nc.sync.dma_start(out=wq_sb[:, 2], in_=wq[2])
            nc.sync.dma_start(out=wk_sb[:, 2], in_=wk[2])
            for c in range(NDC):
                nc.sync.dma_start(out=wv_sb[:, c], in_=wv[:, c])
            for c in range(NDC):
                nc.sync.dma_start(out=xv_sb[:, c], in_=xv[:, c])
            nc.sync.dma_start(out=wq_sb[:, 3], in_=wq[3])
            nc.sync.dma_start(out=wk_sb[:, 3], in_=wk[3])
            nc.sync.dma_start(out=wo_sb[:], in_=wo[:, :])

            # ---- setup compute (runs during initial DMA wait) ----
            # warmup matmuls first: cover the first x/w DMA wait and ramp the
            # PE clock (idle PE restarts at half speed for ~3us).
            scr = cst.tile([P, 640], F16, tag="scr")
            nc.vector.memset(scr[:], 0.001)
            wtb = cst.tile([1, 16], F32, tag="wtb")
            nc.vector.memset(wtb[:], 0.0)
            nc.scalar.activation(wtb[:], wtb[:], EXP, scale=1.0)  # table preload
            pswu = psP.tile([P, 512], F32, tag="p", name="pswu")
            for i in range(12):
                nc.tensor.matmul(pswu[:], scr[:, 0:128], scr[:, 128:640],
                                 start=(i == 0), stop=(i == 11))
            # V-slot ones columns, pre-masked: masked keys drop out of the
            # softmax denominator. On GpSimd so DVE stays clear at startup.
            for h in range(G):
                nc.gpsimd.tensor_copy(vp[:, :, h, DH], vm[:, :])

            # keep-alive matmuls fill DMA-paced stalls in phase A so the PE
            # clock never drops out of its high p-state. They write a psV
            # bank that attention doesn't touch until long after phase A.
            wps = psV.tile([P, 512], F32, tag="v", name="wps")

            def warm_one():
                nc.tensor.matmul(wps[:], scr[:, 0:128], scr[:, 128:640],
                                 start=True, stop=True)

            def warm(n, name=None):
                for _ in range(n):
                    warm_one()

            def mixed(items, nwarm, k):
                # yield items, inserting one keep-alive after every k
                cnt, left = 0, nwarm
                for f in items:
                    yield f
                    cnt += 1
                    if cnt % k == 0 and left > 0:
                        left -= 1
                        yield warm_one

            # ---- emission helpers ----
            def proj_q(t):
                for half in range(2):
                    qsl = slice(half * 512, (half + 1) * 512)
                    psq = psP.tile([P, 512], F32, tag="p", name=f"psq{t}_{half}")
                    for c in range(NDC):
                        yield lambda t=t, c=c, half=half, psq=psq: nc.tensor.matmul(
                            psq[:], wq_sb[:, t, c, :], xq_sb[:, half, c, :],
                            start=(c == 0), stop=(c == NDC - 1))
                    if with_bq:
                        yield lambda t=t, qsl=qsl, psq=psq: nc.vector.tensor_scalar_add(
                            qts[t][:, qsl], psq[:], bq_sb[:, t:t + 1])
                    else:
                        yield lambda t=t, qsl=qsl, psq=psq: nc.scalar.copy(
                            qts[t][:, qsl], psq[:])

            def proj_k(t):
                for s0 in range(0, SK, 512):
                    w = min(512, SK - s0)
                    psk = psP.tile([P, w], F32, tag="p", name=f"psk{t}_{s0}")
                    for c in range(NDC):
                        yield lambda t=t, c=c, s0=s0, w=w, psk=psk: nc.tensor.matmul(
                            psk[:, :w], wk_sb[:, t, c, :], xk_sb[:, c, s0:s0 + w],
                            start=(c == 0), stop=(c == NDC - 1))
                    if with_bk:
                        yield lambda t=t, s0=s0, w=w, psk=psk: nc.vector.tensor_scalar_add(
                            kts[t][:, s0:s0 + w], psk[:, :w], bk_sb[:, t:t + 1])
                    else:
                        yield lambda t=t, s0=s0, w=w, psk=psk: nc.vector.tensor_copy(
                            kts[t][:, s0:s0 + w], psk[:, :w])

            def proj_v():
                for st in range(SKT):
                    psv = psP.tile([P, GC], F32, tag="p", name=f"psv{st}")
                    for c in range(NDC):
                        yield lambda st=st, c=c, psv=psv: nc.tensor.matmul(
                            psv[:], xv_sb[:, c, st * P:(st + 1) * P], wv_sb[:, c, :],
                            start=(c == 0), stop=(c == NDC - 1 and not with_bv))
                    if with_bv:
                        yield lambda psv=psv: nc.tensor.matmul(
                            psv[:], ones1[:], bv_sb[:], start=False, stop=True)
                    # scatter per-head slices into 65-wide V slots, applying
                    # the key mask as a per-partition scale (zeroes masked
                    # key rows out of the numerator).
                    def scatter(st=st, psv=psv):
                        with nc.allow_low_precision("V scatter + key mask at fp16"):
                            nc.vector.tensor_scalar_mul(
                                vp[:, st, :, 0:DH],
                                psv[:].rearrange("p (g d) -> p g d", g=G),
                                vm[:, st:st + 1])
                    yield scatter

            def scores_group(th, st, pte, pto):
                ksl = slice(st * P, (st + 1) * P)
                pse = psS.tile([P, S], F32, tag="s", name=f"pse{th}_{st}")
                pso = psS.tile([P, S], F32, tag="s", name=f"pso{th}_{st}")
                for half in range(2):
                    qsl = slice(half * 512, (half + 1) * 512)
                    nc.tensor.matmul(pse[:, qsl], kts[th][0:64, ksl],
                                     qts[th][0:64, qsl], start=True, stop=True)
                for half in range(2):
                    qsl = slice(half * 512, (half + 1) * 512)
                    nc.tensor.matmul(pso[:, qsl], kts[th][64:128, ksl],
                                     qts[th][64:128, qsl], start=True, stop=True)
                with nc.allow_low_precision("attn weights at fp16"):
                    nc.scalar.activation(pte[:, st, :], pse[:], EXP, scale=0.125)
                    nc.scalar.activation(pto[:, st, :], pso[:], EXP, scale=0.125)

            def attnv(th, pte, pto, tail=False):
                """Returns (gen of attnV matmuls + psV drains, [finish0,
                finish1]). Finishes (pr matmuls + normalize multiply) are
                deferred so their recip-chain wait never head-of-line blocks
                the PE queue."""
                finishes = []

                def gen():
                    for half in range(2):
                        qsl = slice(half * 512, (half + 1) * 512)
                        nve = psV.tile([P, 512], F32, tag="v",
                                       name=f"nve{th}_{half}")
                        nvo = psV.tile([P, 512], F32, tag="v",
                                       name=f"nvo{th}_{half}")
                        for st in range(SKT):
                            yield lambda st=st, nve=nve, qsl=qsl: nc.tensor.matmul(
                                nve[0:VW, :], vp[:, st, 2 * th, :],
                                pte[:, st, qsl],
                                start=(st == 0), stop=(st == SKT - 1))
                        for st in range(SKT):
                            yield lambda st=st, nvo=nvo, qsl=qsl: nc.tensor.matmul(
                                nvo[0:VW, :], vp[:, st, 2 * th + 1, :],
                                pto[:, st, qsl],
                                start=(st == 0), stop=(st == SKT - 1))

                        def drain(half=half, qsl=qsl, nve=nve, nvo=nvo):
                            # psV -> mgs / SBUF fast; gather the 1024
                            # denominators onto 16 partitions so the exact
                            # reciprocal runs at free-size 128; scatter back.
                            dsb = npool.tile([P, 2, 512], F32, tag="dsb",
                                             name=f"dsb{th}_{half}")
                            rsg = npool.tile([P, 64], F32, tag="rsg",
                                             name=f"rsg{th}_{half}")
                            rcp = npool.tile([P, 64], F16, tag="rcp",
                                             name=f"rcp{th}_{half}")
                            rrt = npool.tile([P, 512], F16, tag="rst",
                                             name=f"rrt{th}_{half}")
                            # denominator rows first so the recip chain's
                            # latency isn't queued behind the mgs copies
                            nc.vector.tensor_copy(dsb[64:65, 0, :],
                                                  nve[64:65, :])
                            nc.scalar.copy(dsb[64:65, 1, :], nvo[64:65, :])
                            nc.sync.dma_start(out=rsg[64:80, :],
                                              in_=dsb[64:65, :, :])
                            with nc.allow_low_precision("denom recip fp16"):
                                nc.vector.reciprocal(rcp[64:80, :], rsg[64:80, :])
                            # scatter: partition 64 <- even head's 512 recips,
                            # partition 65 <- odd head's (DMA can cross
                            # partitions even though engines cannot)
                            nc.sync.dma_start(out=rrt[64:66, :],
                                              in_=rcp[64:80, :])
                            with nc.allow_low_precision("numerators at fp16"):
                                if tail:
                                    # ACT is idle in the tail; split the
                                    # mgs copies so the normalize multiply
                                    # lands sooner for phase C
                                    nc.scalar.copy(mgs[th][0:64, qsl],
                                                   nve[0:64, :])
                                else:
                                    nc.vector.tensor_copy(mgs[th][0:64, qsl],
                                                          nve[0:64, :])
                                nc.vector.tensor_copy(mgs[th][64:128, qsl],
                                                      nvo[0:64, :])

                            def finish(qsl=qsl, rrt=rrt, pool=None):
                                pr = (pool or psP).tile(
                                    [P, 512], F32,
                                    tag="p" if pool is None else "v",
                                    name=f"pr{th}_{half}")
                                nc.tensor.matmul(pr[:], es[64:66, :],
                                                 rrt[64:66, :],
                                                 start=True, stop=True)
                                with nc.allow_low_precision("normalize fp16"):
                                    nc.vector.tensor_mul(mgs[th][:, qsl],
                                                         mgs[th][:, qsl], pr[:])
                            finishes.append(finish)
                        yield drain
                return gen(), finishes

            def emit(gen):
                for f in gen:
                    f()

            def weave(score_sts, fillers, per=6):
                """Emit score st-groups paced by `per` filler ops between."""
                fit = iter(fillers)
                for sg in score_sts:
                    sg()
                    for _ in range(per):
                        f = next(fit, None)
                        if f is None:
                            break
                        f()
                for f in fit:
                    f()

            ptes = []
            # th0 prerequisites: keep-alives plug DMA-paced stalls
            emit(mixed(proj_q(0), 9, 2))
            emit(mixed(proj_k(0), 9, 2))
            warm(5)

            def score_region(th):
                pte = pp.tile([P, SKT, S], F16, tag="pe", name=f"pte{th}")
                pto = pp.tile([P, SKT, S], F16, tag="po", name=f"pto{th}")
                ptes.append((pte, pto))
                return [lambda st=st, th=th, pte=pte, pto=pto:
                        scores_group(th, st, pte, pto) for st in range(SKT)]

            # scores th0 woven with Q1/K1 projection (+ keep-alives: both the
            # scores and the projections are DMA/ACT-paced early in R0)
            weave(score_region(0),
                  mixed(itertools.chain(proj_q(1), proj_k(1)), 10, 4), per=7)
            warm(3)   # region boundary: ACT drains its exp backlog
            # scores th1 woven with Q2/K2
            weave(score_region(1), list(proj_q(2)) + list(proj_k(2)), per=6)
            warm(3)
            # scores th2 woven with Q3/K3
            weave(score_region(2), list(proj_q(3)) + list(proj_k(3)), per=6)
            warm(3)
            # V projection burst + attnV th0
            emit(proj_v())
            g0, f0 = attnv(0, *ptes[0])
            emit(g0)
            # scores th3 woven with th0 finishes, attnV th1, and the front of
            # attnV th2 (R3 is ACT-bound: the extra PE filler keeps the clock
            # hot while the exp stream drains). chain() keeps every pool
            # allocation at fire time, preserving slot-rotation order.
            g1, f1 = attnv(1, *ptes[1])
            g2, f2 = attnv(2, *ptes[2], tail=True)
            weave(score_region(3),
                  itertools.chain(f0, g1, itertools.islice(g2, 12)), per=7)
            emit(f1)
            emit(g2)          # remainder of attnV th2
            g3, f3 = attnv(3, *ptes[3], tail=True)
            emit(g3)          # PE stays busy while th2's recip chains run
            for f in f2:      # psV prs: fire-time allocation keeps slot order
                f(pool=psV)

            # ---- output projection ----
            copy_k = [0]
            open_pso = {}

            def c_start(qt_i, oh):
                sqsl = slice(qt_i * P, (qt_i + 1) * P)
                osl = slice(oh * 512, (oh + 1) * 512)
                pso = psP.tile([P, 512], F32, tag="p", name=f"psO{qt_i}_{oh}")
                for c in range(NTH - 1):
                    nc.tensor.matmul(pso[:], mgs[c][:, sqsl], wo_sb[:, c, osl],
                                     start=(c == 0), stop=False)
                open_pso[(qt_i, oh)] = pso

            def c_end(qt_i, oh):
                sqsl = slice(qt_i * P, (qt_i + 1) * P)
                osl = slice(oh * 512, (oh + 1) * 512)
                pso = open_pso.pop((qt_i, oh))
                nc.tensor.matmul(pso[:], mgs[NTH - 1][:, sqsl],
                                 wo_sb[:, NTH - 1, osl], start=False, stop=True)
                ob = ot.tile([P, 512], F32, tag="ob", name=f"ob{qt_i}_{oh}")
                if qt_i == 7:
                    # final tiles: split the drain copy across engines to
                    # shorten the last copy->DMA serial chain
                    nc.vector.tensor_copy(ob[0:64, :], pso[0:64, :])
                    nc.scalar.copy(ob[64:128, :], pso[64:128, :])
                elif copy_k[0] % 2 == 0:
                    nc.vector.tensor_copy(ob[:], pso[:])
                else:
                    nc.scalar.copy(ob[:], pso[:])
                copy_k[0] += 1
                nc.sync.dma_start(out=out[sqsl, osl], in_=ob[:])

            def c_chain(qt_i, oh):
                c_start(qt_i, oh)
                c_end(qt_i, oh)

            # first two chains open on th0-2 while the last normalize's
            # reciprocal chain completes off-PE
            c_start(0, 0)
            c_start(0, 1)
            f3[0](pool=psV)
            c_end(0, 0)
            c_end(0, 1)
            for qt_i in range(1, 4):
                c_chain(qt_i, 0)
                c_chain(qt_i, 1)
            f3[1](pool=psV)
            for qt_i in range(4, 8):
                c_chain(qt_i, 0)
                c_chain(qt_i, 1)

    nc.finalize()
    return nc


_CACHE = {}


def kernel(**inputs):
    queries = np.asarray(inputs["queries"], np.float32)
    keys = np.asarray(inputs["keys"], np.float32)
    values = np.asarray(inputs["values"], np.float32)
    valid_lens = np.asarray(inputs["valid_lens"], np.int32)
    W_q = np.asarray(inputs["W_q"], np.float32)
    W_k = np.asarray(inputs["W_k"], np.float32)
    W_v = np.asarray(inputs["W_v"], np.float32)
    W_o = np.asarray(inputs["W_o"], np.float32)
    b_q = np.asarray(inputs["b_q"], np.float32)
    b_k = np.asarray(inputs["b_k"], np.float32)
    b_v = np.asarray(inputs["b_v"], np.float32)
    b_o = np.asarray(inputs["b_o"], np.float32)

    maxv = int(valid_lens.max())
    SKT = max(1, min(8, -(-maxv // P)))
    SK = SKT * P
    with_bq, with_bk, with_bv = bool(b_q.any()), bool(b_k.any()), bool(b_v.any())

    key = (SKT, with_bq, with_bk, with_bv)
    if key not in _CACHE:
        _CACHE[key] = build(SKT, with_bq, with_bk, with_bv)
    nc = _CACHE[key]

    esel = np.zeros((2, P), np.float16)
    esel[0, 0:DH] = 1.0
    esel[1, DH:2 * DH] = 1.0

    col = np.arange(SK)
    in_maps = []
    for c in range(8):
        b, g = c // 2, c % 2
        gsl = slice(g * GC, (g + 1) * GC)
        WqT = np.ascontiguousarray(W_q.T[:, gsl])         # [D, GC]
        WkT = np.ascontiguousarray(W_k.T[:, gsl])
        WvT = np.ascontiguousarray(W_v.T[:, gsl])
        WoT = np.ascontiguousarray(W_o.T[gsl, :])         # [GC, D]
        Xq, Xk, Xv = queries[b], keys[b], values[b]       # [S, D]
        in_maps.append({
            "xq": np.ascontiguousarray(
                Xq.reshape(2, 512, NDC, P).transpose(0, 3, 2, 1)).astype(np.float16),
            "xk": np.ascontiguousarray(
                Xk[0:SK].reshape(SK, NDC, P).transpose(2, 1, 0)).astype(np.float16),
            "xv": np.ascontiguousarray(
                Xv[0:SK].reshape(SK, NDC, P).transpose(2, 1, 0)).astype(np.float16),
            "wq": np.ascontiguousarray(
                WqT.reshape(NDC, P, NTH, P).transpose(2, 1, 0, 3)).astype(np.float16),
            "wk": np.ascontiguousarray(
                WkT.reshape(NDC, P, NTH, P).transpose(2, 1, 0, 3)).astype(np.float16),
            "wv": np.ascontiguousarray(
                WvT.reshape(NDC, P, GC).transpose(1, 0, 2)).astype(np.float16),
            "wo": np.ascontiguousarray(
                WoT.reshape(NTH, P, D).transpose(1, 0, 2)).astype(np.float16),
            "vmk": np.ascontiguousarray(
                (col < valid_lens[b]).astype(np.float32).reshape(SKT, P).T),
            "esel": esel,
            "bq": np.ascontiguousarray(b_q[gsl]),
            "bk": np.ascontiguousarray(b_k[gsl]),
            "bv": np.ascontiguousarray(b_v[gsl]).astype(np.float16),
        })

    res = run_bass_kernel_spmd(nc, in_maps, list(range(8)))
    final = np.empty((B, S, D), np.float32)
    for b in range(B):
        final[b] = res.results[2 * b]["out"] + res.results[2 * b + 1]["out"] + b_o
    return final
